# revision 28
# baseline (speedup 1.0000x reference)
"""Trainium2 Bass kernel for masked attention scoring (sparse_attention).

Computes, per batch b:
    proj = y @ M^T                      # [B, D]
    eij  = tanh(einsum('bsd,bd->bs', x, proj))
    a    = exp(eij) * mask
    a    = a / (sum_s a + EPS)

Sharding: data-parallel over batch B=32 across 8 NeuronCores (4 batches
per core). M is replicated; all reductions stay local per shard.

Per-core device algorithm (memory-bound, x-stream dominated; the 2e-2
rel-err budget is spent on f16 inputs, keeping end-to-end error ~1e-3):
  - host marshalling: x, y^T, M^T are shipped as f16 (y/M additionally
    pre-transposed), so the device does zero transposes/casts for the
    proj GEMM and the dominant x stream is 16.8 MiB instead of 33.5.
  - proj = yT^T @ M^T accumulated in PSUM f32 on TensorE, then
    broadcast across the 128 partitions via selector matmuls (row-b
    one-hot lhsT), all off the DVE critical path.
  - main pass: stream x in [128, 8, 1024] f16 tiles (natural layout,
    2 MiB DMAs at line rate) and compute the d-reduction per s-chunk,
    split across engines to balance load: 1 in 3 chunks as a fused
    scalar_tensor_tensor(mult, mult, accum_out) on VectorE, the rest
    as a 2x-mode tensor_mul on VectorE + activation(Copy, accum_out)
    reduce on ScalarE.
  - epilogue: tanh+exp per batch on ScalarE as each batch finishes;
    then mask multiply, free-dim reduce, partition reduce + denominator
    broadcast via tiny TensorE matmuls with ones/selectors, normalize,
    PE-transpose, one contiguous DMA out. No strided elementwise DMAs
    anywhere (mask in and a out go through PE transposes).
"""

import os
import sys

import numpy as np

for _p in ("/opt/trn_rl_repo",):
    if os.path.isdir(_p) and _p not in sys.path:
        sys.path.insert(0, _p)

B, S, D = 32, 2048, 1024
NCORES = 8
BL = B // NCORES        # batches per core
P = 128                 # SBUF partitions
J = S // P              # 16 s-chunks per batch
HALF = J // 2           # s-chunks per x DMA (2 MiB in f16)
DC = D // P             # 8 d-chunks
EPS = 1e-7

_CACHE = {}


def _build():
    import concourse.bacc as bacc
    import concourse.tile as tile
    from concourse import mybir
    from concourse.masks import make_identity
    from concourse.tile import add_dep_helper

    f32 = mybir.dt.float32
    f16 = mybir.dt.float16
    i32 = mybir.dt.int32

    nc = bacc.Bacc("TRN2", target_bir_lowering=False, debug=False,
                   num_devices=NCORES)

    x_ext = nc.dram_tensor("x16", [BL, S, D], f16, kind="ExternalInput").ap()
    y_ext = nc.dram_tensor("yT16", [D, BL], f16, kind="ExternalInput").ap()
    mask_ext = nc.dram_tensor("mask", [BL, S], i32, kind="ExternalInput").ap()
    m_ext = nc.dram_tensor("MT16", [D, D], f16, kind="ExternalInput").ap()
    out_ext = nc.dram_tensor("out", [BL, S], f32, kind="ExternalOutput").ap()

    with tile.TileContext(nc) as tc:
        with (
            tc.tile_pool(name="consts", bufs=1) as consts,
            tc.tile_pool(name="psum_t", bufs=2, space="PSUM") as psum_t_pool,
            tc.tile_pool(name="psum_proj", bufs=1, space="PSUM") as psum_proj_pool,
            tc.tile_pool(name="psum_pb", bufs=1, space="PSUM") as psum_pb_pool,
            tc.tile_pool(name="psum_small", bufs=1, space="PSUM") as psum_small_pool,
            tc.tile_pool(name="xpool", bufs=8) as xpool,
            tc.tile_pool(name="scr", bufs=6) as scr_pool,
        ):
            identity16 = consts.tile([P, P], f16)
            make_identity(nc, identity16)
            identity32 = consts.tile([P, P], f32)
            make_identity(nc, identity32)
            ones_col = consts.tile([P, 1], f32)
            nc.vector.memset(ones_col, 1.0)
            ones_row = consts.tile([1, P], f32)
            nc.vector.memset(ones_row, 1.0)
            eps_t = consts.tile([1, 1], f32)
            nc.vector.memset(eps_t, EPS)

            # ---- M^T ships pre-transposed f16 from the host ----
            # mtsb[p_dd, dc, e] = M[e, dc*128+p_dd]; one contiguous DMA
            mtsb = consts.tile([P, DC, D], f16)
            m_dmas = [nc.sync.dma_start(
                out=mtsb,
                in_=m_ext.rearrange("(dc p) e -> p dc e", p=P))]

            # ---- y^T ships pre-transposed f16 from the host ----
            yT = consts.tile([P, DC, BL], f16)
            nc.sync.dma_start(
                out=yT, in_=y_ext.rearrange("(dc p) b -> p dc b", p=P))

            # ---- proj[b, e] = sum_d y[b, d] * M[e, d]  (PSUM f32) ----
            proj_ps = psum_proj_pool.tile([BL, D], f32)
            for dc in range(DC):
                for eh in range(2):
                    nc.tensor.matmul(
                        proj_ps[:, eh * 512:(eh + 1) * 512],
                        lhsT=yT[:, dc, :],
                        rhs=mtsb[:, dc, eh * 512:(eh + 1) * 512],
                        start=(dc == 0),
                        stop=(dc == DC - 1),
                    )
            proj_sb = consts.tile([BL, D], f16)
            nc.vector.tensor_copy(proj_sb, proj_ps)

            # ---- broadcast proj rows across partitions via TensorE ----
            projbc = []
            for b in range(BL):
                sel = consts.tile([BL, P], f16, name=f"sel{b}")
                nc.gpsimd.memset(sel, 0.0)
                nc.gpsimd.affine_select(
                    out=sel, in_=sel,
                    compare_op=mybir.AluOpType.not_equal,
                    fill=1.0, base=-b,
                    pattern=[[0, P]], channel_multiplier=1)
                pb = consts.tile([P, D], f16, name=f"projbc{b}")
                for eh in range(2):
                    pb_ps = psum_pb_pool.tile([P, 512], f32, tag="pbps")
                    nc.tensor.matmul(
                        pb_ps,
                        lhsT=sel,
                        rhs=proj_sb[:, eh * 512:(eh + 1) * 512],
                        start=True, stop=True)
                    if b == 0:
                        nc.vector.tensor_copy(
                            pb[:, eh * 512:(eh + 1) * 512], pb_ps)
                    else:
                        nc.scalar.copy(pb[:, eh * 512:(eh + 1) * 512], pb_ps)
                projbc.append(pb)

            # ---- masks: one contiguous cast-DMA + PE transposes ----
            mk_nat = consts.tile([J, BL, P], f32)
            nc.gpsimd.dma_start(
                out=mk_nat,
                in_=mask_ext.rearrange("b (j p) -> j b p", p=P))
            mask_all = consts.tile([P, BL, J], f32)
            for b in range(BL):
                mk_ps = psum_small_pool.tile([P, J], f32, tag="small")
                nc.tensor.transpose(mk_ps, mk_nat[:, b, :], identity32[:J, :J])
                nc.scalar.copy(mask_all[:, b, :], mk_ps)

            # ---- main pass: eij[p, b, col] = x[b, s, :] . proj[b, :] ----
            eij = consts.tile([P, BL, J], f32)
            th = consts.tile([P, BL, J], f32)
            ex = consts.tile([P, BL, J], f32)
            first_x_dma = None
            for b in range(BL):
                for half in range(2):
                    xt = xpool.tile([P, HALF, D], f16, tag="xt")
                    xd = nc.sync.dma_start(
                        out=xt,
                        in_=x_ext[b, half * HALF * P:(half + 1) * HALF * P, :]
                        .rearrange("(j p) d -> p j d", p=P),
                    )
                    if first_x_dma is None:
                        first_x_dma = xd
                    for j in range(HALF):
                        col = half * HALF + j
                        kind = ("S", "T", "G", "S", "T", "G", "S", "T")[j]
                        if kind == "S":
                            # fused multiply+reduce on DVE
                            scr = scr_pool.tile([P, D], f16, tag="scr")
                            nc.vector.scalar_tensor_tensor(
                                out=scr,
                                in0=xt[:, j, :],
                                scalar=1.0,
                                in1=projbc[b],
                                op0=mybir.AluOpType.mult,
                                op1=mybir.AluOpType.mult,
                                accum_out=eij[:, b, col:col + 1],
                            )
                        elif kind == "T":
                            # 2x-mode multiply on DVE, reduce on ScalarE
                            scr = scr_pool.tile([P, D], f16, tag="scr")
                            nc.vector.tensor_mul(scr, xt[:, j, :],
                                                 projbc[b])
                            dump = scr_pool.tile([P, D], f16, tag="dump",
                                                 bufs=4)
                            nc.scalar.activation(
                                dump, scr,
                                mybir.ActivationFunctionType.Copy,
                                accum_out=eij[:, b, col:col + 1])
                        else:
                            # multiply on GpSimd, reduce on ScalarE
                            gscr = scr_pool.tile([P, D], f16, tag="gscr",
                                                 bufs=3)
                            nc.gpsimd.tensor_mul(gscr, xt[:, j, :],
                                                 projbc[b])
                            gdump = scr_pool.tile([P, D], f16, tag="gdump",
                                                  bufs=2)
                            nc.scalar.activation(
                                gdump, gscr,
                                mybir.ActivationFunctionType.Copy,
                                accum_out=eij[:, b, col:col + 1])
                nc.scalar.activation(th[:, b, :], eij[:, b, :],
                                     mybir.ActivationFunctionType.Tanh)
                nc.scalar.activation(ex[:, b, :], th[:, b, :],
                                     mybir.ActivationFunctionType.Exp)

            # ---- fused epilogue over all batches ----
            au = consts.tile([P, BL, J], f32)
            nc.vector.tensor_mul(au, ex, mask_all)
            cs = consts.tile([P, BL], f32)
            nc.vector.reduce_sum(cs, au, axis=mybir.AxisListType.X)
            tot_ps = psum_small_pool.tile([1, BL], f32, tag="small")
            nc.tensor.matmul(tot_ps, lhsT=ones_col, rhs=cs,
                             start=True, stop=True)
            tot_sb = consts.tile([1, BL], f32)
            nc.scalar.activation(tot_sb, tot_ps,
                                 mybir.ActivationFunctionType.Identity,
                                 bias=eps_t, scale=1.0)
            rec = consts.tile([1, BL], f32)
            nc.vector.reciprocal(rec, tot_sb)
            rbc_ps = psum_small_pool.tile([P, BL], f32, tag="small")
            nc.tensor.matmul(rbc_ps, lhsT=ones_row, rhs=rec,
                             start=True, stop=True)
            rbc_sb = consts.tile([P, BL], f32)
            nc.scalar.copy(rbc_sb, rbc_ps)
            an = consts.tile([P, BL, J], f32)
            for b in range(BL):
                nc.scalar.mul(an[:, b, :], au[:, b, :], rbc_sb[:, b:b + 1])
            at_ps = psum_small_pool.tile([BL * J, P], f32, tag="small")
            nc.tensor.transpose(at_ps, an.rearrange("p b j -> p (b j)"),
                                identity32)
            an_t = consts.tile([BL * J, P], f32)
            nc.scalar.copy(an_t, at_ps)
            nc.sync.dma_start(
                out=out_ext.rearrange("b (j p) -> (b j) p", p=P), in_=an_t)

    nc.compile()
    return nc


def _get_nc():
    if "nc" not in _CACHE:
        _CACHE["nc"] = _build()
    return _CACHE["nc"]


def _in_maps(x, y, mask, M):
    x16 = np.ascontiguousarray(
        np.asarray(x, dtype=np.float32).astype(np.float16))
    y16 = np.asarray(y, dtype=np.float32).astype(np.float16)
    mask = np.ascontiguousarray(np.asarray(mask, dtype=np.int32))
    MT16 = np.ascontiguousarray(np.asarray(M, dtype=np.float32)
                                .astype(np.float16).T)
    return [
        {
            "x16": x16[i * BL:(i + 1) * BL],
            "yT16": np.ascontiguousarray(y16[i * BL:(i + 1) * BL].T),
            "mask": mask[i * BL:(i + 1) * BL],
            "MT16": MT16,
        }
        for i in range(NCORES)
    ]


def kernel(x, y, mask, M, **_ignored):
    from concourse.bass_utils import run_bass_kernel_spmd

    nc = _get_nc()
    res = run_bass_kernel_spmd(nc, _in_maps(x, y, mask, M),
                               core_ids=list(range(NCORES)))
    out = np.concatenate([res.results[i]["out"] for i in range(NCORES)],
                         axis=0)
    return out.astype(np.float32)


# revision 29
# speedup vs baseline: 1.1869x; 1.1869x over previous
"""Trainium2 Bass kernel for masked attention scoring (sparse_attention).

Computes, per batch b:
    proj = y @ M^T                      # [B, D]
    eij  = tanh(einsum('bsd,bd->bs', x, proj))
    a    = exp(eij) * mask
    a    = a / (sum_s a + EPS)

Sharding: data-parallel over batch B=32 across 8 NeuronCores (4 batches
per core). M is replicated; all reductions stay local per shard.

Per-core device algorithm (memory-bound, x-stream dominated; the 2e-2
rel-err budget is spent on f16 inputs, keeping end-to-end error ~1e-3):
  - host marshalling: x, y^T, M^T are shipped as f16 (y/M additionally
    pre-transposed), so the device does zero transposes/casts for the
    proj GEMM and the dominant x stream is 16.8 MiB instead of 33.5.
  - proj = yT^T @ M^T accumulated in PSUM f32 on TensorE, then
    broadcast across the 128 partitions via selector matmuls (row-b
    one-hot lhsT), all off the DVE critical path.
  - main pass: stream x in [128, 8, 1024] f16 tiles (natural layout,
    2 MiB DMAs at line rate) and compute the d-reduction per s-chunk,
    split across engines to balance load: 1 in 3 chunks as a fused
    scalar_tensor_tensor(mult, mult, accum_out) on VectorE, the rest
    as a 2x-mode tensor_mul on VectorE + activation(Copy, accum_out)
    reduce on ScalarE.
  - epilogue: tanh+exp per batch on ScalarE as each batch finishes;
    then mask multiply, free-dim reduce, partition reduce + denominator
    broadcast via tiny TensorE matmuls with ones/selectors, normalize,
    PE-transpose, one contiguous DMA out. No strided elementwise DMAs
    anywhere (mask in and a out go through PE transposes).
"""

import os
import sys

import numpy as np

for _p in ("/opt/trn_rl_repo",):
    if os.path.isdir(_p) and _p not in sys.path:
        sys.path.insert(0, _p)

B, S, D = 32, 2048, 1024
NCORES = 8
BL = B // NCORES        # batches per core
P = 128                 # SBUF partitions
J = S // P              # 16 s-chunks per batch
HALF = J // 2           # s-chunks per x DMA (2 MiB in f16)
DC = D // P             # 8 d-chunks
EPS = 1e-7

_CACHE = {}


def _build():
    import concourse.bacc as bacc
    import concourse.tile as tile
    from concourse import mybir
    from concourse.masks import make_identity
    from concourse.tile import add_dep_helper

    f32 = mybir.dt.float32
    f16 = mybir.dt.float16
    i32 = mybir.dt.int32

    nc = bacc.Bacc("TRN2", target_bir_lowering=False, debug=False,
                   num_devices=NCORES)

    x_ext = nc.dram_tensor("x16", [BL, S, D], f16, kind="ExternalInput").ap()
    y_ext = nc.dram_tensor("yT16", [D, BL], f16, kind="ExternalInput").ap()
    mask_ext = nc.dram_tensor("mask", [BL, S], i32, kind="ExternalInput").ap()
    m_ext = nc.dram_tensor("MT16", [D, D], f16, kind="ExternalInput").ap()
    out_ext = nc.dram_tensor("out", [BL, S], f32, kind="ExternalOutput").ap()

    with tile.TileContext(nc) as tc:
        with (
            tc.tile_pool(name="consts", bufs=1) as consts,
            tc.tile_pool(name="psum_t", bufs=2, space="PSUM") as psum_t_pool,
            tc.tile_pool(name="psum_proj", bufs=1, space="PSUM") as psum_proj_pool,
            tc.tile_pool(name="psum_pb", bufs=1, space="PSUM") as psum_pb_pool,
            tc.tile_pool(name="psum_small", bufs=1, space="PSUM") as psum_small_pool,
            tc.tile_pool(name="xpool", bufs=8) as xpool,
            tc.tile_pool(name="scr", bufs=6) as scr_pool,
        ):
            identity16 = consts.tile([P, P], f16)
            make_identity(nc, identity16)
            identity32 = consts.tile([P, P], f32)
            make_identity(nc, identity32)
            ones_col = consts.tile([P, 1], f32)
            nc.vector.memset(ones_col, 1.0)
            ones_row = consts.tile([1, P], f32)
            nc.vector.memset(ones_row, 1.0)
            eps_t = consts.tile([1, 1], f32)
            nc.vector.memset(eps_t, EPS)

            # ---- M^T ships pre-transposed f16 from the host ----
            # mtsb[p_dd, dc, e] = M[e, dc*128+p_dd]; one contiguous DMA
            mtsb = consts.tile([P, DC, D], f16)
            m_dmas = [nc.sync.dma_start(
                out=mtsb,
                in_=m_ext.rearrange("(dc p) e -> p dc e", p=P))]

            # ---- y^T ships pre-transposed f16 from the host ----
            yT = consts.tile([P, DC, BL], f16)
            nc.sync.dma_start(
                out=yT, in_=y_ext.rearrange("(dc p) b -> p dc b", p=P))

            # ---- proj[b, e] = sum_d y[b, d] * M[e, d]  (PSUM f32) ----
            proj_ps = psum_proj_pool.tile([BL, D], f32)
            for dc in range(DC):
                for eh in range(2):
                    nc.tensor.matmul(
                        proj_ps[:, eh * 512:(eh + 1) * 512],
                        lhsT=yT[:, dc, :],
                        rhs=mtsb[:, dc, eh * 512:(eh + 1) * 512],
                        start=(dc == 0),
                        stop=(dc == DC - 1),
                    )
            proj_sb = consts.tile([BL, D], f16)
            nc.vector.tensor_copy(proj_sb, proj_ps)

            # ---- broadcast proj rows across partitions via TensorE ----
            projbc = []
            for b in range(BL):
                sel = consts.tile([BL, P], f16, name=f"sel{b}")
                nc.gpsimd.memset(sel, 0.0)
                nc.gpsimd.affine_select(
                    out=sel, in_=sel,
                    compare_op=mybir.AluOpType.not_equal,
                    fill=1.0, base=-b,
                    pattern=[[0, P]], channel_multiplier=1)
                pb = consts.tile([P, D], f16, name=f"projbc{b}")
                for eh in range(2):
                    pb_ps = psum_pb_pool.tile([P, 512], f32, tag="pbps")
                    nc.tensor.matmul(
                        pb_ps,
                        lhsT=sel,
                        rhs=proj_sb[:, eh * 512:(eh + 1) * 512],
                        start=True, stop=True)
                    if b == 0:
                        nc.vector.tensor_copy(
                            pb[:, eh * 512:(eh + 1) * 512], pb_ps)
                    else:
                        nc.scalar.copy(pb[:, eh * 512:(eh + 1) * 512], pb_ps)
                projbc.append(pb)

            # ---- masks: one contiguous cast-DMA + PE transposes ----
            mk_nat = consts.tile([J, BL, P], f32)
            nc.gpsimd.dma_start(
                out=mk_nat,
                in_=mask_ext.rearrange("b (j p) -> j b p", p=P))
            mask_all = consts.tile([P, BL, J], f32)
            for b in range(BL):
                mk_ps = psum_small_pool.tile([P, J], f32, tag="small")
                nc.tensor.transpose(mk_ps, mk_nat[:, b, :], identity32[:J, :J])
                nc.scalar.copy(mask_all[:, b, :], mk_ps)

            # ---- main pass: eij[p, b, col] = x[b, s, :] . proj[b, :] ----
            eij = consts.tile([P, BL, J], f32)
            th = consts.tile([P, BL, J], f32)
            ex = consts.tile([P, BL, J], f32)
            first_x_dma = None
            for b in range(BL):
                for half in range(2):
                    xt = xpool.tile([P, HALF, D], f16, tag="xt")
                    xd = nc.sync.dma_start(
                        out=xt,
                        in_=x_ext[b, half * HALF * P:(half + 1) * HALF * P, :]
                        .rearrange("(j p) d -> p j d", p=P),
                    )
                    if first_x_dma is None:
                        first_x_dma = xd
                    for j in range(HALF):
                        col = half * HALF + j
                        scr = scr_pool.tile([P, D], f16, tag="scr")
                        if col % 3 == 2:
                            # fused multiply+reduce on DVE
                            nc.vector.scalar_tensor_tensor(
                                out=scr,
                                in0=xt[:, j, :],
                                scalar=1.0,
                                in1=projbc[b],
                                op0=mybir.AluOpType.mult,
                                op1=mybir.AluOpType.mult,
                                accum_out=eij[:, b, col:col + 1],
                            )
                        else:
                            # 2x-mode multiply on DVE, reduce on ScalarE
                            nc.vector.tensor_mul(scr, xt[:, j, :],
                                                 projbc[b])
                            dump = scr_pool.tile([P, D], f16, tag="dump",
                                                 bufs=4)
                            nc.scalar.activation(
                                dump, scr,
                                mybir.ActivationFunctionType.Copy,
                                accum_out=eij[:, b, col:col + 1])
                nc.scalar.activation(th[:, b, :], eij[:, b, :],
                                     mybir.ActivationFunctionType.Tanh)
                nc.scalar.activation(ex[:, b, :], th[:, b, :],
                                     mybir.ActivationFunctionType.Exp)

            # ---- fused epilogue over all batches ----
            au = consts.tile([P, BL, J], f32)
            nc.vector.tensor_mul(au, ex, mask_all)
            cs = consts.tile([P, BL], f32)
            nc.vector.reduce_sum(cs, au, axis=mybir.AxisListType.X)
            tot_ps = psum_small_pool.tile([1, BL], f32, tag="small")
            nc.tensor.matmul(tot_ps, lhsT=ones_col, rhs=cs,
                             start=True, stop=True)
            tot_sb = consts.tile([1, BL], f32)
            nc.scalar.activation(tot_sb, tot_ps,
                                 mybir.ActivationFunctionType.Identity,
                                 bias=eps_t, scale=1.0)
            rec = consts.tile([1, BL], f32)
            nc.vector.reciprocal(rec, tot_sb)
            rbc_ps = psum_small_pool.tile([P, BL], f32, tag="small")
            nc.tensor.matmul(rbc_ps, lhsT=ones_row, rhs=rec,
                             start=True, stop=True)
            rbc_sb = consts.tile([P, BL], f32)
            nc.scalar.copy(rbc_sb, rbc_ps)
            an = consts.tile([P, BL, J], f32)
            for b in range(BL):
                nc.scalar.mul(an[:, b, :], au[:, b, :], rbc_sb[:, b:b + 1])
            at_ps = psum_small_pool.tile([BL * J, P], f32, tag="small")
            nc.tensor.transpose(at_ps, an.rearrange("p b j -> p (b j)"),
                                identity32)
            an_t = consts.tile([BL * J, P], f32)
            nc.scalar.copy(an_t, at_ps)
            nc.sync.dma_start(
                out=out_ext.rearrange("b (j p) -> (b j) p", p=P), in_=an_t)

    nc.compile()
    return nc


def _get_nc():
    if "nc" not in _CACHE:
        _CACHE["nc"] = _build()
    return _CACHE["nc"]


def _in_maps(x, y, mask, M):
    x16 = np.ascontiguousarray(
        np.asarray(x, dtype=np.float32).astype(np.float16))
    y16 = np.asarray(y, dtype=np.float32).astype(np.float16)
    mask = np.ascontiguousarray(np.asarray(mask, dtype=np.int32))
    MT16 = np.ascontiguousarray(np.asarray(M, dtype=np.float32)
                                .astype(np.float16).T)
    return [
        {
            "x16": x16[i * BL:(i + 1) * BL],
            "yT16": np.ascontiguousarray(y16[i * BL:(i + 1) * BL].T),
            "mask": mask[i * BL:(i + 1) * BL],
            "MT16": MT16,
        }
        for i in range(NCORES)
    ]


def kernel(x, y, mask, M, **_ignored):
    from concourse.bass_utils import run_bass_kernel_spmd

    nc = _get_nc()
    res = run_bass_kernel_spmd(nc, _in_maps(x, y, mask, M),
                               core_ids=list(range(NCORES)))
    out = np.concatenate([res.results[i]["out"] for i in range(NCORES)],
                         axis=0)
    return out.astype(np.float32)


# revision 30
# speedup vs baseline: 1.1902x; 1.0028x over previous
"""Trainium2 Bass kernel for masked attention scoring (sparse_attention).

Computes, per batch b:
    proj = y @ M^T                      # [B, D]
    eij  = tanh(einsum('bsd,bd->bs', x, proj))
    a    = exp(eij) * mask
    a    = a / (sum_s a + EPS)

Sharding: data-parallel over batch B=32 across 8 NeuronCores (4 batches
per core). M is replicated; all reductions stay local per shard.

Per-core device algorithm (memory-bound, x-stream dominated; the 2e-2
rel-err budget is spent on f16 inputs, keeping end-to-end error ~1e-3):
  - host marshalling: x, y^T, M^T are shipped as f16 (y/M additionally
    pre-transposed), so the device does zero transposes/casts for the
    proj GEMM and the dominant x stream is 16.8 MiB instead of 33.5.
  - proj = yT^T @ M^T accumulated in PSUM f32 on TensorE, then
    broadcast across the 128 partitions via selector matmuls (row-b
    one-hot lhsT), all off the DVE critical path.
  - main pass: stream x in [128, 8, 1024] f16 tiles (natural layout,
    2 MiB DMAs at line rate) and compute the d-reduction per s-chunk,
    split across engines to balance load: 1 in 3 chunks as a fused
    scalar_tensor_tensor(mult, mult, accum_out) on VectorE, the rest
    as a 2x-mode tensor_mul on VectorE + activation(Copy, accum_out)
    reduce on ScalarE.
  - epilogue: tanh+exp per batch on ScalarE as each batch finishes;
    then mask multiply, free-dim reduce, partition reduce + denominator
    broadcast via tiny TensorE matmuls with ones/selectors, normalize,
    PE-transpose, one contiguous DMA out. No strided elementwise DMAs
    anywhere (mask in and a out go through PE transposes).
"""

import os
import sys

import numpy as np

for _p in ("/opt/trn_rl_repo",):
    if os.path.isdir(_p) and _p not in sys.path:
        sys.path.insert(0, _p)

B, S, D = 32, 2048, 1024
NCORES = 8
BL = B // NCORES        # batches per core
P = 128                 # SBUF partitions
J = S // P              # 16 s-chunks per batch
HALF = J // 2           # s-chunks per x DMA (2 MiB in f16)
DC = D // P             # 8 d-chunks
EPS = 1e-7

_CACHE = {}


def _build():
    import concourse.bacc as bacc
    import concourse.tile as tile
    from concourse import mybir
    from concourse.masks import make_identity
    from concourse.tile import add_dep_helper

    f32 = mybir.dt.float32
    f16 = mybir.dt.float16
    i32 = mybir.dt.int32

    nc = bacc.Bacc("TRN2", target_bir_lowering=False, debug=False,
                   num_devices=NCORES)

    x_ext = nc.dram_tensor("x16", [BL, S, D], f16, kind="ExternalInput").ap()
    y_ext = nc.dram_tensor("yT16", [D, BL], f16, kind="ExternalInput").ap()
    mask_ext = nc.dram_tensor("mask", [BL, S], i32, kind="ExternalInput").ap()
    m_ext = nc.dram_tensor("MT16", [D, D], f16, kind="ExternalInput").ap()
    out_ext = nc.dram_tensor("out", [BL, S], f32, kind="ExternalOutput").ap()

    with tile.TileContext(nc) as tc:
        with (
            tc.tile_pool(name="consts", bufs=1) as consts,
            tc.tile_pool(name="psum_t", bufs=2, space="PSUM") as psum_t_pool,
            tc.tile_pool(name="psum_proj", bufs=1, space="PSUM") as psum_proj_pool,
            tc.tile_pool(name="psum_pb", bufs=1, space="PSUM") as psum_pb_pool,
            tc.tile_pool(name="psum_small", bufs=1, space="PSUM") as psum_small_pool,
            tc.tile_pool(name="xpool", bufs=8) as xpool,
            tc.tile_pool(name="scr", bufs=6) as scr_pool,
        ):
            identity16 = consts.tile([P, P], f16)
            make_identity(nc, identity16)
            identity32 = consts.tile([P, P], f32)
            make_identity(nc, identity32)
            ones_col = consts.tile([P, 1], f32)
            nc.vector.memset(ones_col, 1.0)
            ones_row = consts.tile([1, P], f32)
            nc.vector.memset(ones_row, 1.0)
            eps_t = consts.tile([1, 1], f32)
            nc.vector.memset(eps_t, EPS)

            # ---- M^T ships pre-transposed f16 from the host ----
            # mtsb[p_dd, dc, e] = M[e, dc*128+p_dd]; one contiguous DMA
            mtsb = consts.tile([P, DC, D], f16)
            m_src = m_ext.rearrange("(dc p) e -> p dc e", p=P)
            m_dmas = [
                nc.sync.dma_start(out=mtsb[:, 0:DC // 2, :],
                                  in_=m_src[:, 0:DC // 2, :]),
                nc.sync.dma_start(out=mtsb[:, DC // 2:, :],
                                  in_=m_src[:, DC // 2:, :]),
            ]

            # ---- y^T ships pre-transposed f16 from the host ----
            yT = consts.tile([P, DC, BL], f16)
            nc.sync.dma_start(
                out=yT, in_=y_ext.rearrange("(dc p) b -> p dc b", p=P))

            # ---- proj[b, e] = sum_d y[b, d] * M[e, d]  (PSUM f32) ----
            proj_ps = psum_proj_pool.tile([BL, D], f32)
            for dc in range(DC):
                for eh in range(2):
                    nc.tensor.matmul(
                        proj_ps[:, eh * 512:(eh + 1) * 512],
                        lhsT=yT[:, dc, :],
                        rhs=mtsb[:, dc, eh * 512:(eh + 1) * 512],
                        start=(dc == 0),
                        stop=(dc == DC - 1),
                    )
            proj_sb = consts.tile([BL, D], f16)
            nc.vector.tensor_copy(proj_sb, proj_ps)

            # ---- broadcast proj rows across partitions via TensorE ----
            projbc = []
            for b in range(BL):
                sel = consts.tile([BL, P], f16, name=f"sel{b}")
                nc.gpsimd.memset(sel, 0.0)
                nc.gpsimd.affine_select(
                    out=sel, in_=sel,
                    compare_op=mybir.AluOpType.not_equal,
                    fill=1.0, base=-b,
                    pattern=[[0, P]], channel_multiplier=1)
                pb = consts.tile([P, D], f16, name=f"projbc{b}")
                for eh in range(2):
                    pb_ps = psum_pb_pool.tile([P, 512], f32, tag="pbps")
                    nc.tensor.matmul(
                        pb_ps,
                        lhsT=sel,
                        rhs=proj_sb[:, eh * 512:(eh + 1) * 512],
                        start=True, stop=True)
                    if b == 0:
                        nc.vector.tensor_copy(
                            pb[:, eh * 512:(eh + 1) * 512], pb_ps)
                    else:
                        nc.scalar.copy(pb[:, eh * 512:(eh + 1) * 512], pb_ps)
                projbc.append(pb)

            # ---- masks: one contiguous cast-DMA + PE transposes ----
            mk_nat = consts.tile([J, BL, P], f32)
            nc.gpsimd.dma_start(
                out=mk_nat,
                in_=mask_ext.rearrange("b (j p) -> j b p", p=P))
            mask_all = consts.tile([P, BL, J], f32)
            for b in range(BL):
                mk_ps = psum_small_pool.tile([P, J], f32, tag="small")
                nc.tensor.transpose(mk_ps, mk_nat[:, b, :], identity32[:J, :J])
                nc.scalar.copy(mask_all[:, b, :], mk_ps)

            # ---- main pass: eij[p, b, col] = x[b, s, :] . proj[b, :] ----
            eij = consts.tile([P, BL, J], f32)
            th = consts.tile([P, BL, J], f32)
            ex = consts.tile([P, BL, J], f32)
            first_x_dma = None
            for b in range(BL):
                for half in range(2):
                    xt = xpool.tile([P, HALF, D], f16, tag="xt")
                    xd = nc.sync.dma_start(
                        out=xt,
                        in_=x_ext[b, half * HALF * P:(half + 1) * HALF * P, :]
                        .rearrange("(j p) d -> p j d", p=P),
                    )
                    if first_x_dma is None:
                        first_x_dma = xd
                    for j in range(HALF):
                        col = half * HALF + j
                        scr = scr_pool.tile([P, D], f16, tag="scr")
                        if col % 3 == 2:
                            # fused multiply+reduce on DVE
                            nc.vector.scalar_tensor_tensor(
                                out=scr,
                                in0=xt[:, j, :],
                                scalar=1.0,
                                in1=projbc[b],
                                op0=mybir.AluOpType.mult,
                                op1=mybir.AluOpType.mult,
                                accum_out=eij[:, b, col:col + 1],
                            )
                        else:
                            # 2x-mode multiply on DVE, reduce on ScalarE
                            nc.vector.tensor_mul(scr, xt[:, j, :],
                                                 projbc[b])
                            dump = scr_pool.tile([P, D], f16, tag="dump",
                                                 bufs=4)
                            nc.scalar.activation(
                                dump, scr,
                                mybir.ActivationFunctionType.Copy,
                                accum_out=eij[:, b, col:col + 1])
            # ---- fused epilogue over all batches ----
            nc.scalar.activation(th, eij, mybir.ActivationFunctionType.Tanh)
            nc.scalar.activation(ex, th, mybir.ActivationFunctionType.Exp)
            au = consts.tile([P, BL, J], f32)
            nc.vector.tensor_mul(au, ex, mask_all)
            cs = consts.tile([P, BL], f32)
            nc.vector.reduce_sum(cs, au, axis=mybir.AxisListType.X)
            tot_ps = psum_small_pool.tile([1, BL], f32, tag="small")
            nc.tensor.matmul(tot_ps, lhsT=ones_col, rhs=cs,
                             start=True, stop=True)
            tot_sb = consts.tile([1, BL], f32)
            nc.scalar.activation(tot_sb, tot_ps,
                                 mybir.ActivationFunctionType.Identity,
                                 bias=eps_t, scale=1.0)
            rec = consts.tile([1, BL], f32)
            nc.vector.reciprocal(rec, tot_sb)
            rbc_ps = psum_small_pool.tile([P, BL], f32, tag="small")
            nc.tensor.matmul(rbc_ps, lhsT=ones_row, rhs=rec,
                             start=True, stop=True)
            rbc_sb = consts.tile([P, BL], f32)
            nc.scalar.copy(rbc_sb, rbc_ps)
            an = consts.tile([P, BL, J], f32)
            for b in range(BL):
                nc.scalar.mul(an[:, b, :], au[:, b, :], rbc_sb[:, b:b + 1])
            at_ps = psum_small_pool.tile([BL * J, P], f32, tag="small")
            nc.tensor.transpose(at_ps, an.rearrange("p b j -> p (b j)"),
                                identity32)
            an_t = consts.tile([BL * J, P], f32)
            nc.scalar.copy(an_t, at_ps)
            nc.sync.dma_start(
                out=out_ext.rearrange("b (j p) -> (b j) p", p=P), in_=an_t)

    nc.compile()
    return nc


def _get_nc():
    if "nc" not in _CACHE:
        _CACHE["nc"] = _build()
    return _CACHE["nc"]


def _in_maps(x, y, mask, M):
    x16 = np.ascontiguousarray(
        np.asarray(x, dtype=np.float32).astype(np.float16))
    y16 = np.asarray(y, dtype=np.float32).astype(np.float16)
    mask = np.ascontiguousarray(np.asarray(mask, dtype=np.int32))
    MT16 = np.ascontiguousarray(np.asarray(M, dtype=np.float32)
                                .astype(np.float16).T)
    return [
        {
            "x16": x16[i * BL:(i + 1) * BL],
            "yT16": np.ascontiguousarray(y16[i * BL:(i + 1) * BL].T),
            "mask": mask[i * BL:(i + 1) * BL],
            "MT16": MT16,
        }
        for i in range(NCORES)
    ]


def kernel(x, y, mask, M, **_ignored):
    from concourse.bass_utils import run_bass_kernel_spmd

    nc = _get_nc()
    res = run_bass_kernel_spmd(nc, _in_maps(x, y, mask, M),
                               core_ids=list(range(NCORES)))
    out = np.concatenate([res.results[i]["out"] for i in range(NCORES)],
                         axis=0)
    return out.astype(np.float32)


# revision 31
# speedup vs baseline: 1.2560x; 1.0553x over previous
"""Trainium2 Bass kernel for masked attention scoring (sparse_attention).

Computes, per batch b:
    proj = y @ M^T                      # [B, D]
    eij  = tanh(einsum('bsd,bd->bs', x, proj))
    a    = exp(eij) * mask
    a    = a / (sum_s a + EPS)

Sharding: data-parallel over batch B=32 across 8 NeuronCores (4 batches
per core). M is replicated; all reductions stay local per shard.

Per-core device algorithm (memory-bound, x-stream dominated; the 2e-2
rel-err budget is spent on f16 inputs, keeping end-to-end error ~1e-3):
  - host marshalling: x, y^T, M^T are shipped as f16 (y/M additionally
    pre-transposed), so the device does zero transposes/casts for the
    proj GEMM and the dominant x stream is 16.8 MiB instead of 33.5.
  - proj = yT^T @ M^T accumulated in PSUM f32 on TensorE, then
    broadcast across the 128 partitions via selector matmuls (row-b
    one-hot lhsT), all off the DVE critical path.
  - main pass: stream x in [128, 8, 1024] f16 tiles (natural layout,
    2 MiB DMAs at line rate) and compute the d-reduction per s-chunk,
    split across engines to balance load: 1 in 3 chunks as a fused
    scalar_tensor_tensor(mult, mult, accum_out) on VectorE, the rest
    as a 2x-mode tensor_mul on VectorE + activation(Copy, accum_out)
    reduce on ScalarE.
  - epilogue: tanh+exp per batch on ScalarE as each batch finishes;
    then mask multiply, free-dim reduce, partition reduce + denominator
    broadcast via tiny TensorE matmuls with ones/selectors, normalize,
    PE-transpose, one contiguous DMA out. No strided elementwise DMAs
    anywhere (mask in and a out go through PE transposes).
"""

import os
import sys

import numpy as np

for _p in ("/opt/trn_rl_repo",):
    if os.path.isdir(_p) and _p not in sys.path:
        sys.path.insert(0, _p)

B, S, D = 32, 2048, 1024
NCORES = 8
BL = B // NCORES        # batches per core
P = 128                 # SBUF partitions
J = S // P              # 16 s-chunks per batch
HALF = J // 2           # s-chunks per x DMA (2 MiB in f16)
DC = D // P             # 8 d-chunks
EPS = 1e-7

_CACHE = {}


def _build():
    import concourse.bacc as bacc
    import concourse.tile as tile
    from concourse import mybir
    from concourse.masks import make_identity
    from concourse.tile import add_dep_helper

    f32 = mybir.dt.float32
    f16 = mybir.dt.float16
    i32 = mybir.dt.int32

    nc = bacc.Bacc("TRN2", target_bir_lowering=False, debug=False,
                   num_devices=NCORES)

    x_ext = nc.dram_tensor("x16", [BL, S, D], f16, kind="ExternalInput").ap()
    y_ext = nc.dram_tensor("yT16", [D, BL], f16, kind="ExternalInput").ap()
    mask_ext = nc.dram_tensor("mask", [BL, S], i32, kind="ExternalInput").ap()
    m_ext = nc.dram_tensor("MT16", [D, D], f16, kind="ExternalInput").ap()
    out_ext = nc.dram_tensor("out", [BL, S], f32, kind="ExternalOutput").ap()

    with tile.TileContext(nc) as tc:
        with (
            tc.tile_pool(name="consts", bufs=1) as consts,
            tc.tile_pool(name="psum_t", bufs=2, space="PSUM") as psum_t_pool,
            tc.tile_pool(name="psum_proj", bufs=1, space="PSUM") as psum_proj_pool,
            tc.tile_pool(name="psum_pb", bufs=1, space="PSUM") as psum_pb_pool,
            tc.tile_pool(name="psum_small", bufs=1, space="PSUM") as psum_small_pool,
            tc.tile_pool(name="xpool", bufs=8) as xpool,
            tc.tile_pool(name="scr", bufs=6) as scr_pool,
        ):
            identity16 = consts.tile([P, P], f16)
            make_identity(nc, identity16)
            identity32 = consts.tile([P, P], f32)
            make_identity(nc, identity32)
            ones_col = consts.tile([P, 1], f32)
            nc.vector.memset(ones_col, 1.0)
            ones_row = consts.tile([1, P], f32)
            nc.vector.memset(ones_row, 1.0)
            eps_t = consts.tile([1, 1], f32)
            nc.vector.memset(eps_t, EPS)

            # ---- M^T ships pre-transposed f16 from the host ----
            # mtsb[p_dd, dc, e] = M[e, dc*128+p_dd]; one contiguous DMA
            mtsb = consts.tile([P, DC, D], f16)
            m_src = m_ext.rearrange("(dc p) e -> p dc e", p=P)
            m_dmas = [
                nc.sync.dma_start(out=mtsb[:, 0:DC // 2, :],
                                  in_=m_src[:, 0:DC // 2, :]),
                nc.sync.dma_start(out=mtsb[:, DC // 2:, :],
                                  in_=m_src[:, DC // 2:, :]),
            ]

            # warm the PE clock (1.2 -> 2.4 GHz needs ~4us sustained)
            warm_ps = psum_t_pool.tile([P, P], f16, tag="warm")
            for _ in range(12):
                nc.tensor.transpose(warm_ps, identity16, identity16)

            # ---- y^T ships pre-transposed f16 from the host ----
            yT = consts.tile([P, DC, BL], f16)
            nc.sync.dma_start(
                out=yT, in_=y_ext.rearrange("(dc p) b -> p dc b", p=P))

            # ---- proj[b, e] = sum_d y[b, d] * M[e, d]  (PSUM f32) ----
            proj_ps = psum_proj_pool.tile([BL, D], f32)
            proj_mms = []
            for dc in range(DC):
                for eh in range(2):
                    proj_mms.append(nc.tensor.matmul(
                        proj_ps[:, eh * 512:(eh + 1) * 512],
                        lhsT=yT[:, dc, :],
                        rhs=mtsb[:, dc, eh * 512:(eh + 1) * 512],
                        start=(dc == 0),
                        stop=(dc == DC - 1),
                    ))
            proj_sb = consts.tile([BL, D], f16)
            nc.vector.tensor_copy(proj_sb[:, 0:512], proj_ps[:, 0:512])
            nc.vector.tensor_copy(proj_sb[:, 512:], proj_ps[:, 512:])

            # ---- broadcast proj rows across partitions via TensorE ----
            projbc = []
            for b in range(BL):
                sel = consts.tile([BL, P], f16, name=f"sel{b}")
                nc.gpsimd.memset(sel, 0.0)
                nc.gpsimd.affine_select(
                    out=sel, in_=sel,
                    compare_op=mybir.AluOpType.not_equal,
                    fill=1.0, base=-b,
                    pattern=[[0, P]], channel_multiplier=1)
                pb = consts.tile([P, D], f16, name=f"projbc{b}")
                for eh in range(2):
                    pb_ps = psum_pb_pool.tile([P, 512], f32, tag="pbps")
                    nc.tensor.matmul(
                        pb_ps,
                        lhsT=sel,
                        rhs=proj_sb[:, eh * 512:(eh + 1) * 512],
                        start=True, stop=True)
                    if b == 0:
                        nc.vector.tensor_copy(
                            pb[:, eh * 512:(eh + 1) * 512], pb_ps)
                    else:
                        nc.scalar.copy(pb[:, eh * 512:(eh + 1) * 512], pb_ps)
                projbc.append(pb)

            # ---- masks: one contiguous cast-DMA + PE transposes ----
            mk_nat = consts.tile([J, BL, P], f32)
            nc.gpsimd.dma_start(
                out=mk_nat,
                in_=mask_ext.rearrange("b (j p) -> j b p", p=P))
            mask_all = consts.tile([P, BL, J], f32)
            for b in range(BL):
                mk_ps = psum_small_pool.tile([P, J], f32, tag="small")
                mk_t = nc.tensor.transpose(mk_ps, mk_nat[:, b, :],
                                           identity32[:J, :J])
                add_dep_helper(mk_t.ins, proj_mms[-1].ins, sync=False,
                               reason="mask transposes after proj GEMM")
                nc.scalar.copy(mask_all[:, b, :], mk_ps)

            # ---- main pass: eij[p, b, col] = x[b, s, :] . proj[b, :] ----
            eij = consts.tile([P, BL, J], f32)
            th = consts.tile([P, BL, J], f32)
            ex = consts.tile([P, BL, J], f32)
            first_x_dma = None
            for b in range(BL):
                for half in range(2):
                    xt = xpool.tile([P, HALF, D], f16, tag="xt")
                    xd = nc.sync.dma_start(
                        out=xt,
                        in_=x_ext[b, half * HALF * P:(half + 1) * HALF * P, :]
                        .rearrange("(j p) d -> p j d", p=P),
                    )
                    if first_x_dma is None:
                        first_x_dma = xd
                    for j in range(HALF):
                        col = half * HALF + j
                        scr = scr_pool.tile([P, D], f16, tag="scr")
                        if col % 3 == 2 or (col == 0 and b % 2 == 1):
                            # fused multiply+reduce on DVE
                            nc.vector.scalar_tensor_tensor(
                                out=scr,
                                in0=xt[:, j, :],
                                scalar=1.0,
                                in1=projbc[b],
                                op0=mybir.AluOpType.mult,
                                op1=mybir.AluOpType.mult,
                                accum_out=eij[:, b, col:col + 1],
                            )
                        else:
                            # 2x-mode multiply on DVE, reduce on ScalarE
                            nc.vector.tensor_mul(scr, xt[:, j, :],
                                                 projbc[b])
                            dump = scr_pool.tile([P, D], f16, tag="dump",
                                                 bufs=4)
                            nc.scalar.activation(
                                dump, scr,
                                mybir.ActivationFunctionType.Copy,
                                accum_out=eij[:, b, col:col + 1])
            # ---- fused epilogue over all batches ----
            nc.scalar.activation(th, eij, mybir.ActivationFunctionType.Tanh)
            nc.scalar.activation(ex, th, mybir.ActivationFunctionType.Exp)
            au = consts.tile([P, BL, J], f32)
            nc.vector.tensor_mul(au, ex, mask_all)
            cs = consts.tile([P, BL], f32)
            nc.vector.reduce_sum(cs, au, axis=mybir.AxisListType.X)
            tot_ps = psum_small_pool.tile([1, BL], f32, tag="small")
            nc.tensor.matmul(tot_ps, lhsT=ones_col, rhs=cs,
                             start=True, stop=True)
            tot_sb = consts.tile([1, BL], f32)
            nc.scalar.activation(tot_sb, tot_ps,
                                 mybir.ActivationFunctionType.Identity,
                                 bias=eps_t, scale=1.0)
            rec = consts.tile([1, BL], f32)
            nc.vector.reciprocal(rec, tot_sb)
            rbc_ps = psum_small_pool.tile([P, BL], f32, tag="small")
            nc.tensor.matmul(rbc_ps, lhsT=ones_row, rhs=rec,
                             start=True, stop=True)
            rbc_sb = consts.tile([P, BL], f32)
            nc.scalar.copy(rbc_sb, rbc_ps)
            an = consts.tile([P, BL, J], f32)
            for b in range(BL):
                nc.scalar.mul(an[:, b, :], au[:, b, :], rbc_sb[:, b:b + 1])
            at_ps = psum_small_pool.tile([BL * J, P], f32, tag="small")
            nc.tensor.transpose(at_ps, an.rearrange("p b j -> p (b j)"),
                                identity32)
            an_t = consts.tile([BL * J, P], f32)
            nc.scalar.copy(an_t, at_ps)
            nc.sync.dma_start(
                out=out_ext.rearrange("b (j p) -> (b j) p", p=P), in_=an_t)

    nc.compile()
    return nc


def _get_nc():
    if "nc" not in _CACHE:
        _CACHE["nc"] = _build()
    return _CACHE["nc"]


def _in_maps(x, y, mask, M):
    x16 = np.ascontiguousarray(
        np.asarray(x, dtype=np.float32).astype(np.float16))
    y16 = np.asarray(y, dtype=np.float32).astype(np.float16)
    mask = np.ascontiguousarray(np.asarray(mask, dtype=np.int32))
    MT16 = np.ascontiguousarray(np.asarray(M, dtype=np.float32)
                                .astype(np.float16).T)
    return [
        {
            "x16": x16[i * BL:(i + 1) * BL],
            "yT16": np.ascontiguousarray(y16[i * BL:(i + 1) * BL].T),
            "mask": mask[i * BL:(i + 1) * BL],
            "MT16": MT16,
        }
        for i in range(NCORES)
    ]


def kernel(x, y, mask, M, **_ignored):
    from concourse.bass_utils import run_bass_kernel_spmd

    nc = _get_nc()
    res = run_bass_kernel_spmd(nc, _in_maps(x, y, mask, M),
                               core_ids=list(range(NCORES)))
    out = np.concatenate([res.results[i]["out"] for i in range(NCORES)],
                         axis=0)
    return out.astype(np.float32)
